# revision 17
# baseline (speedup 1.0000x reference)
"""Contrastive loss kernel for Trainium2 (8 NeuronCores).

loss = mean((sim.sum(-1) - diag) / T) with sim = n @ n.T, n = x/||x||
     = (||sum_i n_i||^2 - sum_i ||n_i||^2) / (N*T)
     = (||s||^2 - N) / (N*T)          with s = sum_i x_i / ||x_i||

Each core takes a [2048, 512] row shard (16 row-tiles of [128, 512]).
Row sum-of-squares alternates between VectorE (bn_stats -> D*(var+mean^2),
even tiles) and ScalarE (Square activation with accum_out, odd tiles) to
balance the engines; rnorm = reciprocal(sqrt(ss)) with the sqrt batched
per group. The partial s_local = sum_i rnorm_i * x_i is 16 PE matmuls
(lhsT = rnorm [128,1], rhs = x tile [128,512], float32r for full-rate PE)
accumulated in one PSUM bank, copied to SBUF, and DMA'd out per core as
a [1, 512] vector. The host sums the 8 partials and applies the scalar
epilogue (all-reduce of a [D] vector + scalar math).
"""

import numpy as np

import concourse.bass as bass
import concourse.bacc as bacc
import concourse.tile as tile
from concourse import mybir
from concourse.bass_utils import run_bass_kernel_spmd

N = 16384
D = 512
NCORES = 8
ROWS = N // NCORES   # 2048 rows per core
P = 128              # SBUF partitions
NTILES = ROWS // P   # 16 row-tiles per core
GROUPS = (4, 4, 2, 2, 1, 1, 1, 1)   # rsqrt batch sizes (sum = NTILES)
TEMPERATURE = 0.5

F32 = mybir.dt.float32
F32R = mybir.dt.float32r
SQUARE = mybir.ActivationFunctionType.Square

_NC = None


def _build_nc() -> bass.Bass:
    nc = bacc.Bacc(None)
    x_in = nc.declare_dram_parameter("x", [ROWS, D], F32R, isOutput=False)
    s_out = nc.declare_dram_parameter("s", [1, D], F32, isOutput=True)
    x_t = x_in.rearrange("(t p) d -> p t d", p=P)

    with tile.TileContext(nc) as tc:
        with (
            tc.tile_pool(name="xs", bufs=NTILES) as xs_pool,
            tc.tile_pool(name="sq", bufs=2) as sq_pool,
            tc.tile_pool(name="bn", bufs=4) as bn_pool,
            tc.tile_pool(name="st", bufs=16) as st_pool,
            tc.tile_pool(name="acc", bufs=1, space="PSUM") as psum_pool,
            tc.tile_pool(name="one", bufs=1) as one_pool,
        ):
            acc = psum_pool.tile([1, D], F32)

            xt = [None] * NTILES   # float32r views (PE operands)
            xf = [None] * NTILES   # float32 views of the same bytes (stats)
            for i in range(NTILES):
                x2 = xs_pool.tile([P, D], F32R)
                nc.sync.dma_start(out=x2, in_=x_t[:, i, :])
                xt[i] = x2[:, :]
                xf[i] = x2[:, :].bitcast(F32)

            def emit_stats(t, ss_col):
                if t % 2 == 1:
                    # ScalarE: ss = sum_d x^2 via Square + accumulate
                    sq = sq_pool.tile([P, D], F32)
                    nc.scalar.activation(
                        out=sq, in_=xf[t], func=SQUARE, accum_out=ss_col
                    )
                else:
                    # VectorE: ss = D*(var + mean^2) == sum_d x^2
                    bn6 = bn_pool.tile([P, 6], F32, tag="bn6")
                    nc.vector.bn_stats(out=bn6, in_=xf[t])
                    mv = bn_pool.tile([P, 2], F32, tag="mv")
                    nc.vector.bn_aggr(out=mv, in_=bn6)
                    m2 = bn_pool.tile([P, 1], F32, tag="m2")
                    nc.vector.tensor_mul(m2, mv[:, 0:1], mv[:, 0:1])
                    nc.vector.tensor_scalar(
                        out=ss_col,
                        in0=m2,
                        scalar1=mv[:, 1:2],
                        scalar2=float(D),
                        op0=mybir.AluOpType.add,
                        op1=mybir.AluOpType.mult,
                    )

            rn = [None] * NTILES
            base = 0
            for gsz in GROUPS:
                tiles = range(base, base + gsz)
                base += gsz
                ss = st_pool.tile([P, gsz], F32, tag="ss")
                for j, t in enumerate(tiles):
                    emit_stats(t, ss[:, j : j + 1])
                nc.scalar.sqrt(out=ss, in_=ss)
                r = st_pool.tile([P, gsz], F32R, tag="rn")
                with nc.allow_low_precision(reason="fp32r rounding for PE operands"):
                    nc.vector.reciprocal(out=r, in_=ss)
                for j, t in enumerate(tiles):
                    rn[t] = r[:, j : j + 1]

            for i in range(NTILES):
                nc.tensor.matmul(
                    acc,
                    lhsT=rn[i],
                    rhs=xt[i],
                    start=(i == 0),
                    stop=(i == NTILES - 1),
                )

            res = one_pool.tile([1, D], F32)
            nc.scalar.copy(out=res, in_=acc)
            nc.sync.dma_start(out=s_out[:, :], in_=res)

    nc.finalize()
    return nc


def _run(x: np.ndarray, trace: bool = False):
    global _NC
    if _NC is None:
        _NC = _build_nc()
    x = np.ascontiguousarray(np.asarray(x, dtype=np.float32)).reshape(NCORES, ROWS, D)
    in_maps = [{"x": x[c]} for c in range(NCORES)]
    out = run_bass_kernel_spmd(_NC, in_maps, core_ids=list(range(NCORES)), trace=trace)
    s = np.zeros(D, dtype=np.float64)
    for r in out.results:
        s += r["s"].reshape(D).astype(np.float64)
    loss = (float(s @ s) - float(N)) / (N * TEMPERATURE)
    return np.asarray(loss, dtype=np.float32), out


def kernel(x: np.ndarray) -> np.ndarray:
    loss, _ = _run(x)
    return loss


# revision 18
# speedup vs baseline: 1.0104x; 1.0104x over previous
"""Contrastive loss kernel for Trainium2 (8 NeuronCores).

loss = mean((sim.sum(-1) - diag) / T) with sim = n @ n.T, n = x/||x||
     = (||sum_i n_i||^2 - sum_i ||n_i||^2) / (N*T)
     = (||s||^2 - N) / (N*T)          with s = sum_i x_i / ||x_i||

Each core takes a [2048, 512] row shard (16 row-tiles of [128, 512]).
Row sum-of-squares alternates between VectorE (bn_stats -> D*(var+mean^2),
even tiles) and ScalarE (Square activation with accum_out, odd tiles) to
balance the engines; rnorm = reciprocal(sqrt(ss)) with the sqrt batched
per group. The partial s_local = sum_i rnorm_i * x_i is 16 PE matmuls
(lhsT = rnorm [128,1], rhs = x tile [128,512], float32r for full-rate PE)
accumulated in one PSUM bank, copied to SBUF, and DMA'd out per core as
a [1, 512] vector. The host sums the 8 partials and applies the scalar
epilogue (all-reduce of a [D] vector + scalar math).
"""

import numpy as np

import concourse.bass as bass
import concourse.bacc as bacc
import concourse.tile as tile
from concourse import mybir
from concourse.bass_utils import run_bass_kernel_spmd

N = 16384
D = 512
NCORES = 8
ROWS = N // NCORES   # 2048 rows per core
P = 128              # SBUF partitions
NTILES = ROWS // P   # 16 row-tiles per core
GROUPS = (4, 4, 2, 1, 1, 1, 1, 1, 1)   # rsqrt batch sizes (sum = NTILES)
TEMPERATURE = 0.5

F32 = mybir.dt.float32
F32R = mybir.dt.float32r
SQUARE = mybir.ActivationFunctionType.Square

_NC = None


def _build_nc() -> bass.Bass:
    nc = bacc.Bacc(None)
    x_in = nc.declare_dram_parameter("x", [ROWS, D], F32R, isOutput=False)
    s_out = nc.declare_dram_parameter("s", [1, D], F32, isOutput=True)
    x_t = x_in.rearrange("(t p) d -> p t d", p=P)

    with tile.TileContext(nc) as tc:
        with (
            tc.tile_pool(name="xs", bufs=NTILES) as xs_pool,
            tc.tile_pool(name="sq", bufs=2) as sq_pool,
            tc.tile_pool(name="bn", bufs=4) as bn_pool,
            tc.tile_pool(name="st", bufs=16) as st_pool,
            tc.tile_pool(name="acc", bufs=1, space="PSUM") as psum_pool,
            tc.tile_pool(name="one", bufs=1) as one_pool,
        ):
            acc = psum_pool.tile([1, D], F32)

            xt = [None] * NTILES   # float32r views (PE operands)
            xf = [None] * NTILES   # float32 views of the same bytes (stats)
            for i in range(NTILES):
                x2 = xs_pool.tile([P, D], F32R)
                nc.sync.dma_start(out=x2, in_=x_t[:, i, :])
                xt[i] = x2[:, :]
                xf[i] = x2[:, :].bitcast(F32)

            def emit_stats(t, ss_col):
                if t % 2 == 1:
                    # ScalarE: ss = sum_d x^2 via Square + accumulate
                    sq = sq_pool.tile([P, D], F32)
                    nc.scalar.activation(
                        out=sq, in_=xf[t], func=SQUARE, accum_out=ss_col
                    )
                else:
                    # VectorE: ss = D*(var + mean^2) == sum_d x^2
                    bn6 = bn_pool.tile([P, 6], F32, tag="bn6")
                    nc.vector.bn_stats(out=bn6, in_=xf[t])
                    mv = bn_pool.tile([P, 2], F32, tag="mv")
                    nc.vector.bn_aggr(out=mv, in_=bn6)
                    m2 = bn_pool.tile([P, 1], F32, tag="m2")
                    nc.vector.tensor_mul(m2, mv[:, 0:1], mv[:, 0:1])
                    nc.vector.tensor_scalar(
                        out=ss_col,
                        in0=m2,
                        scalar1=mv[:, 1:2],
                        scalar2=float(D),
                        op0=mybir.AluOpType.add,
                        op1=mybir.AluOpType.mult,
                    )

            rn = [None] * NTILES
            base = 0
            for gsz in GROUPS:
                tiles = range(base, base + gsz)
                base += gsz
                ss = st_pool.tile([P, gsz], F32, tag="ss")
                for j, t in enumerate(tiles):
                    emit_stats(t, ss[:, j : j + 1])
                nc.scalar.sqrt(out=ss, in_=ss)
                r = st_pool.tile([P, gsz], F32R, tag="rn")
                with nc.allow_low_precision(reason="fp32r rounding for PE operands"):
                    nc.vector.reciprocal(out=r, in_=ss)
                for j, t in enumerate(tiles):
                    rn[t] = r[:, j : j + 1]

            for i in range(NTILES):
                nc.tensor.matmul(
                    acc,
                    lhsT=rn[i],
                    rhs=xt[i],
                    start=(i == 0),
                    stop=(i == NTILES - 1),
                )

            res = one_pool.tile([1, D], F32)
            nc.scalar.copy(out=res, in_=acc)
            nc.sync.dma_start(out=s_out[:, :], in_=res)

    nc.finalize()
    return nc


def _run(x: np.ndarray, trace: bool = False):
    global _NC
    if _NC is None:
        _NC = _build_nc()
    x = np.ascontiguousarray(np.asarray(x, dtype=np.float32)).reshape(NCORES, ROWS, D)
    in_maps = [{"x": x[c]} for c in range(NCORES)]
    out = run_bass_kernel_spmd(_NC, in_maps, core_ids=list(range(NCORES)), trace=trace)
    s = np.zeros(D, dtype=np.float64)
    for r in out.results:
        s += r["s"].reshape(D).astype(np.float64)
    loss = (float(s @ s) - float(N)) / (N * TEMPERATURE)
    return np.asarray(loss, dtype=np.float32), out


def kernel(x: np.ndarray) -> np.ndarray:
    loss, _ = _run(x)
    return loss
